# revision 1
# baseline (speedup 1.0000x reference)
"""Trainium2 Bass kernel v2 for nn_DeformSpaceAttentionv5.

Changes vs v1 baseline:
- Pair-table gather: each descriptor fetches a 2x2 pixel block (1024 fp16 =
  2KB) so SWDGE descriptor count halves (1152/block); desc-gen was the Pool
  bottleneck at ~8ns/desc. Two SWDGE queues alternate per block to overlap
  issue with drain.
- Wide-op bilinear: weights are materialized as a [128, 9, 4, 256] fp16
  tensor (DMA broadcast of [128,36] to 32-wide + 3 DVE doubling copies),
  then ONE big TT-mult + 2 TT-adds + a 4-op max tree replace the per-sample
  scalar_tensor_tensor chains (which run at 1x DVE rate with no fast mode).
- Epilogue matmul/normalized-correlation path unchanged from v1.
"""

import numpy as np

B, C, H, W = 4, 256, 128, 128
PAD = 8
Hp, Wp = H + 2 * PAD, W + 2 * PAD
ROWS = 64            # rows per core (H split in 2)
N = ROWS * W         # positions per core
BLK = 128            # positions per block (= one row)
NBLK = N // BLK      # 64
NIDX = 9 * BLK       # gather indices per block
EPS = 1e-5

_NC_CACHE = {}


def _build_nc(has_bias: bool):
    import concourse.bacc as bacc
    import concourse.bass as bass
    import concourse.tile as tile
    import concourse.mybir as mybir
    from concourse import library_config

    f16 = mybir.dt.float16
    f32 = mybir.dt.float32
    i16 = mybir.dt.int16
    Alu = mybir.AluOpType
    Act = mybir.ActivationFunctionType

    nc = bacc.Bacc("TRN2", target_bir_lowering=False, debug=False, num_devices=8,
                   num_swdge_queues=2)

    NTAB = Hp * Wp
    xt2 = nc.dram_tensor("xt2", [NTAB * 1024], f16, kind="ExternalInput")
    xk = nc.dram_tensor("xk", [2, 128, N], f16, kind="ExternalInput")
    idx = nc.dram_tensor("idx", [NBLK, 128, NIDX // 16], i16, kind="ExternalInput")
    w4 = nc.dram_tensor("w4", [NBLK, 128, 36, 32], f16, kind="ExternalInput")
    w4f = nc.dram_tensor("w4f", [NBLK, 128, 8], f32, kind="ExternalInput")
    w0t = nc.dram_tensor("w0t", [2, 128, 257], f16, kind="ExternalInput")
    w1t = nc.dram_tensor("w1t", [2, 128, 257], f16, kind="ExternalInput")
    idmat = nc.dram_tensor("idmat", [128, 128], f16, kind="ExternalInput")
    if has_bias:
        qb = nc.dram_tensor("qb", [128, 257], f32, kind="ExternalInput")
        kb = nc.dram_tensor("kb", [128, 257], f32, kind="ExternalInput")
    o = nc.dram_tensor("o", [128, NBLK], f32, kind="ExternalOutput")

    # gather view: element j = xt2[j*1024 : (j+1)*1024]  (one 2x2 pixel block)
    xt2_view = bass.AP(tensor=xt2[:].tensor, offset=0, ap=[[1024, NTAB], [1, 1024]])

    with tile.TileContext(nc) as tc:
        import contextlib

        with contextlib.ExitStack() as ctx:
            consts = ctx.enter_context(tc.tile_pool(name="consts", bufs=1))
            iopool = ctx.enter_context(tc.tile_pool(name="io", bufs=3))
            gpool = ctx.enter_context(tc.tile_pool(name="gath", bufs=5))
            work = ctx.enter_context(tc.tile_pool(name="work", bufs=2))
            work3 = ctx.enter_context(tc.tile_pool(name="work3", bufs=3))
            pspool = ctx.enter_context(tc.tile_pool(name="ps", bufs=2, space="PSUM"))

            # constants
            w0t_sb = consts.tile([128, 2, 257], f16)
            nc.sync.dma_start(out=w0t_sb, in_=w0t[:, :, :].rearrange("t p o -> p t o"))
            w1t_sb = consts.tile([128, 2, 257], f16)
            nc.sync.dma_start(out=w1t_sb, in_=w1t[:, :, :].rearrange("t p o -> p t o"))
            ident = consts.tile([128, 128], f16)
            nc.sync.dma_start(out=ident, in_=idmat[:, :])
            if has_bias:
                qb_sb = consts.tile([128, 257], f32)
                nc.sync.dma_start(out=qb_sb, in_=qb[:, :])
                kb_sb = consts.tile([128, 257], f32)
                nc.sync.dma_start(out=kb_sb, in_=kb[:, :])

            # per-block scalar accumulators [128 pos, NBLK]
            sqs = consts.tile([128, NBLK], f32, tag="sqs")
            sks = consts.tile([128, NBLK], f32, tag="sks")
            sqks = consts.tile([128, NBLK], f32, tag="sqks")
            sQs = consts.tile([128, NBLK], f32, tag="sQs")
            sKs = consts.tile([128, NBLK], f32, tag="sKs")

            nc.gpsimd.load_library(library_config.mlp)

            pending = None
            for blk in range(NBLK):
                idx_t = iopool.tile([128, NIDX // 16], i16, tag="idx")
                nc.sync.dma_start(out=idx_t, in_=idx[blk])
                w32t = iopool.tile([128, 36, 32], f16, tag="w32t")
                nc.sync.dma_start(out=w32t, in_=w4[blk])
                w4f_t = iopool.tile([128, 8], f32, tag="w4f_t")
                nc.sync.dma_start(out=w4f_t, in_=w4f[blk])
                xk_t = iopool.tile([128, 2, BLK], f16, tag="xk")
                nc.sync.dma_start(
                    out=xk_t, in_=xk[:, :, blk * BLK:(blk + 1) * BLK]
                    .rearrange("t p n -> p t n")
                )
                gat = gpool.tile([128, 9, 1024], f16, tag="gat")
                nc.gpsimd.dma_gather(
                    gat, xt2_view, idx_t, NIDX, NIDX, 1024,
                    single_packet=False, queue_num=blk % 2,
                )

                # t = gat * W: k=0..6 on DVE (weight broadcast folded in
                # via stride-0 middle dim); k=7,8 corner products on the
                # Scalar engine (per-partition scale = per-position weight)
                t_t = work.tile([128, 9, 4, 256], f16, tag="t")
                nc.vector.tensor_tensor(
                    t_t[:, 0:7, :, :].rearrange("p a b (c d) -> p (a b) c d", c=8),
                    gat[:, 0:7, :].rearrange("p a (b c d) -> p (a b) c d", b=4, c=8),
                    w32t[:, 0:28, None, :].to_broadcast([128, 28, 8, 32]),
                    Alu.mult)
                for kk in (7, 8):
                    for r in range(4):
                        nc.scalar.mul(
                            t_t[:, kk, r, :],
                            gat[:, kk, r * 256:(r + 1) * 256],
                            w4f_t[:, (kk - 7) * 4 + r:(kk - 7) * 4 + r + 1])
                # corner pair adds: A = t[:, :, 0:2] + t[:, :, 2:4] -> [128,9,2,256]
                a_t = work.tile([128, 9, 2, 256], f16, tag="a")
                nc.vector.tensor_tensor(
                    a_t, t_t[:, :, 0:2, :], t_t[:, :, 2:4, :], Alu.add)
                # S = A[:, :, 0] + A[:, :, 1] -> [128, 9, 256]
                s_t = work3.tile([128, 9, 256], f16, tag="s")
                nc.vector.tensor_tensor(
                    s_t, a_t[:, :, 0, :], a_t[:, :, 1, :], Alu.add)
                # max tree over the 9 samples
                m1 = work3.tile([128, 4, 256], f16, tag="m1")
                nc.vector.tensor_tensor(m1, s_t[:, 0:4, :], s_t[:, 4:8, :], Alu.max)
                m2 = work3.tile([128, 2, 256], f16, tag="m2")
                nc.vector.tensor_tensor(m2, m1[:, 0:2, :], m1[:, 2:4, :], Alu.max)
                q_t = work3.tile([128, 256], f16, tag="q")
                nc.vector.tensor_tensor(
                    q_t[:, None, :], m2[:, 0:1, :], m2[:, 1:2, :], Alu.max)
                nc.vector.tensor_tensor(
                    q_t[:, None, :], q_t[:, None, :], s_t[:, 8:9, :], Alu.max)

                if pending is not None:
                    pQ, pK, pcol = pending
                    dve_scr = work3.tile([128, 256], f16, tag="dve_scr")
                    nc.vector.scalar_tensor_tensor(
                        dve_scr, pQ[:, 0:256], 0.0, pK, Alu.bypass, Alu.mult,
                        accum_out=sqks[:, pcol],
                    )
                    pending = None

                # transpose q -> qT (c-major) via PE
                qt_ps = pspool.tile([128, 2, 128], f16, tag="qt")
                for t in range(2):
                    nc.tensor.transpose(
                        qt_ps[:, t, :], q_t[:, t * 128:(t + 1) * 128], ident
                    )
                qt_sb = work3.tile([128, 2, 128], f16, tag="qt_sb")
                nc.scalar.copy(qt_sb, qt_ps)

                # Q = qT^T @ w0t  -> [128 pos, 257] (col 256 = sum_o Q)
                Q_ps = pspool.tile([128, 257], f32, tag="Q")
                for t in range(2):
                    nc.tensor.matmul(
                        Q_ps, qt_sb[:, t, :], w0t_sb[:, t, :],
                        start=(t == 0), stop=(t == 1),
                    )
                K_ps = pspool.tile([128, 257], f32, tag="K")
                for t in range(2):
                    nc.tensor.matmul(
                        K_ps, xk_t[:, t, :],
                        w1t_sb[:, t, :], start=(t == 0), stop=(t == 1),
                    )
                if has_bias:
                    nc.vector.tensor_tensor(Q_ps, Q_ps, qb_sb, Alu.add)
                    nc.vector.tensor_tensor(K_ps, K_ps, kb_sb, Alu.add)

                # epilogue reductions (ACT side in-block; the DVE
                # product-accumulate is deferred one block so it never
                # head-of-line blocks the next block's multiply)
                col = slice(blk, blk + 1)
                act_scr = work3.tile([128, 256], f16, tag="act_scr")
                nc.scalar.activation(
                    act_scr, Q_ps[:, 0:256], Act.Square,
                    accum_out=sqs[:, col],
                )
                K_sb = work3.tile([128, 256], f16, tag="K_sb")
                nc.scalar.copy(K_sb, K_ps[:, 0:256])
                nc.scalar.activation(
                    act_scr, K_sb, Act.Square, accum_out=sks[:, col],
                )
                nc.scalar.copy(sQs[:, col], Q_ps[:, 256:257])
                nc.scalar.copy(sKs[:, col], K_ps[:, 256:257])
                pending = (Q_ps, K_sb, col)

            if pending is not None:
                pQ, pK, pcol = pending
                dve_scr = work3.tile([128, 256], f16, tag="dve_scr")
                nc.vector.scalar_tensor_tensor(
                    dve_scr, pQ[:, 0:256], 0.0, pK, Alu.bypass, Alu.mult,
                    accum_out=sqks[:, pcol],
                )

            # final combine over [128, NBLK]
            tmp = consts.tile([128, NBLK], f32, tag="tmp")
            num = consts.tile([128, NBLK], f32, tag="num")
            dq = consts.tile([128, NBLK], f32, tag="dq")
            dk = consts.tile([128, NBLK], f32, tag="dk")
            out_t = consts.tile([128, NBLK], f32, tag="out")
            inv_c = -1.0 / C
            nc.vector.tensor_tensor(tmp, sQs, sKs, Alu.mult)
            nc.vector.scalar_tensor_tensor(num, tmp, inv_c, sqks, Alu.mult, Alu.add)
            nc.vector.tensor_tensor(tmp, sQs, sQs, Alu.mult)
            nc.vector.scalar_tensor_tensor(dq, tmp, inv_c, sqs, Alu.mult, Alu.add)
            nc.vector.tensor_scalar(dq, dq, EPS, None, Alu.add)
            nc.vector.tensor_tensor(tmp, sKs, sKs, Alu.mult)
            nc.vector.scalar_tensor_tensor(dk, tmp, inv_c, sks, Alu.mult, Alu.add)
            nc.vector.tensor_scalar(dk, dk, EPS, None, Alu.add)
            nc.vector.tensor_tensor(tmp, dq, dk, Alu.mult)
            nc.scalar.activation(tmp, tmp, Act.Sqrt)
            nc.vector.reciprocal(tmp, tmp)
            nc.vector.tensor_tensor(out_t, num, tmp, Alu.mult)
            nc.sync.dma_start(out=o[:, :], in_=out_t)

    nc.compile()
    return nc


def _get_nc(has_bias: bool):
    if has_bias not in _NC_CACHE:
        _NC_CACHE[has_bias] = _build_nc(has_bias)
    return _NC_CACHE[has_bias]


def _build_table(x_b):
    """Pair-table for one image: [Hp, Wp, 4, 256] fp16 flat.
    Entry (y, x) = channels of (y,x), (y,x+1), (y+1,x), (y+1,x+1)."""
    xp = np.zeros((Hp, Wp, C), np.float16)
    xp[PAD:PAD + H, PAD:PAD + W, :] = x_b.transpose(1, 2, 0)
    t = np.zeros((Hp, Wp, 4, C), np.float16)
    t[:-1, :-1, 0] = xp[:-1, :-1]
    t[:-1, :-1, 1] = xp[:-1, 1:]
    t[:-1, :-1, 2] = xp[1:, :-1]
    t[:-1, :-1, 3] = xp[1:, 1:]
    return t.reshape(-1)


def _prep_core(x_b, off_b, h0):
    """Host-side per-core prep: pair-table idx [NBLK,128,72] i16,
    corner weights w4 [NBLK,128,36] f16, xk [2,128,N] f16."""
    ys, xs = np.meshgrid(
        np.arange(h0, h0 + ROWS), np.arange(W), indexing="ij"
    )
    ys = ys.reshape(-1).astype(np.float32)   # [N] position y (row-major blocks)
    xs = xs.reshape(-1).astype(np.float32)

    iy = ys.astype(np.int32)
    ix = xs.astype(np.int32)
    k = np.arange(9)
    kh = (k // 3 - 1).astype(np.float32)     # [9]
    kw = (k % 3 - 1).astype(np.float32)

    offy = off_b[2 * k][:, iy, ix]           # [9, N]
    offx = off_b[2 * k + 1][:, iy, ix]
    py = ys[None, :] + kh[:, None] + offy    # [9, N]
    px = xs[None, :] + kw[:, None] + offx
    y0 = np.clip(np.floor(py).astype(np.int32), -PAD, H + PAD - 2)
    x0 = np.clip(np.floor(px).astype(np.int32), -PAD, W + PAD - 2)
    fy = (py - y0).astype(np.float32)        # [9, N]
    fx = (px - x0).astype(np.float32)
    pidx = (y0 + PAD) * Wp + (x0 + PAD)      # [9, N]

    # idx slots: per block, m = k*128 + pos  -> out[pos, k, :]
    slots = pidx.reshape(9, NBLK, BLK).transpose(1, 0, 2)  # [NBLK, 9, BLK]
    wrapped = slots.reshape(NBLK, NIDX // 16, 16).transpose(0, 2, 1)  # [NBLK,16,72]
    idx_np = np.tile(wrapped, (1, 8, 1)).astype(np.int16)  # [NBLK, 128, 72]

    # corner weights [NBLK, 128, 36, 64] f16 (k-major, corner minor, rep-64)
    gy1 = (1.0 - fy) * (1.0 - fx)
    gy2 = (1.0 - fy) * fx
    gy3 = fy * (1.0 - fx)
    gy4 = fy * fx
    w_all = np.stack([gy1, gy2, gy3, gy4], axis=1)  # [9, 4, N]
    w_all = w_all.reshape(9, 4, NBLK, BLK).transpose(2, 3, 0, 1)  # [NBLK,BLK,9,4]
    w4_np = np.ascontiguousarray(np.broadcast_to(
        w_all.reshape(NBLK, 128, 36, 1).astype(np.float16),
        (NBLK, 128, 36, 32)))
    w4f_np = np.ascontiguousarray(
        w_all.reshape(NBLK, 128, 36)[:, :, 28:36].astype(np.float32))

    xk_np = np.ascontiguousarray(
        x_b.reshape(2, 128, H, W)[:, :, h0:h0 + ROWS, :].reshape(2, 128, N)
    ).astype(np.float16)
    return idx_np, w4_np, w4f_np, xk_np


def prep_in_maps(x, offset, w0, b0, w1, b1, has_bias):
    w0t_np = np.concatenate([w0.T, w0.sum(0)[:, None]], 1).astype(np.float16)
    w1t_np = np.concatenate([w1.T, w1.sum(0)[:, None]], 1).astype(np.float16)
    w0t_np = np.ascontiguousarray(w0t_np.reshape(2, 128, 257))
    w1t_np = np.ascontiguousarray(w1t_np.reshape(2, 128, 257))

    in_maps = []
    xt_cache = {}
    for core in range(8):
        b, half = core // 2, core % 2
        h0 = ROWS * half
        if b not in xt_cache:
            xt_cache[b] = _build_table(x[b])
        idx_np, w4_np, w4f_np, xk_np = _prep_core(x[b], offset[b], h0)
        m = {
            "idmat": np.eye(128, dtype=np.float16),
            "xt2": xt_cache[b],
            "xk": xk_np,
            "idx": idx_np,
            "w4": w4_np,
            "w4f": w4f_np,
            "w0t": w0t_np,
            "w1t": w1t_np,
        }
        if has_bias:
            qb_np = np.concatenate([b0, [b0.sum()]]).astype(np.float32)
            kb_np = np.concatenate([b1, [b1.sum()]]).astype(np.float32)
            m["qb"] = np.tile(qb_np[None, :], (128, 1))
            m["kb"] = np.tile(kb_np[None, :], (128, 1))
        in_maps.append(m)
    return in_maps


def kernel(x, offset, w0, b0, w1, b1):
    from concourse.bass_utils import run_bass_kernel_spmd

    x = np.asarray(x, np.float32)
    offset = np.asarray(offset, np.float32)
    w0 = np.asarray(w0, np.float32)
    w1 = np.asarray(w1, np.float32)
    b0 = np.asarray(b0, np.float32)
    b1 = np.asarray(b1, np.float32)

    has_bias = bool(np.any(b0)) or bool(np.any(b1))
    nc = _get_nc(has_bias)
    in_maps = prep_in_maps(x, offset, w0, b0, w1, b1, has_bias)
    res = run_bass_kernel_spmd(nc, in_maps, core_ids=list(range(8)))

    out = np.empty((B, 1, H, W), np.float32)
    for core in range(8):
        b, half = core // 2, core % 2
        h0 = ROWS * half
        o = res.results[core]["o"]  # [128 pos(x), 64 rows]
        out[b, 0, h0:h0 + ROWS, :] = o.T
    return out



# revision 3
# speedup vs baseline: 4.0788x; 4.0788x over previous
"""Trainium2 Bass kernel v2 for nn_DeformSpaceAttentionv5.

Changes vs v1 baseline:
- Pair-table gather: each descriptor fetches a 2x2 pixel block (1024 fp16 =
  2KB) so SWDGE descriptor count halves (1152/block); desc-gen was the Pool
  bottleneck at ~8ns/desc. Two SWDGE queues alternate per block to overlap
  issue with drain.
- Wide-op bilinear: weights are materialized as a [128, 9, 4, 256] fp16
  tensor (DMA broadcast of [128,36] to 32-wide + 3 DVE doubling copies),
  then ONE big TT-mult + 2 TT-adds + a 4-op max tree replace the per-sample
  scalar_tensor_tensor chains (which run at 1x DVE rate with no fast mode).
- Epilogue matmul/normalized-correlation path unchanged from v1.
"""

import numpy as np

B, C, H, W = 4, 256, 128, 128
PAD = 8
Hp, Wp = H + 2 * PAD, W + 2 * PAD
ROWS = 64            # rows per core (H split in 2)
N = ROWS * W         # positions per core
BLK = 128            # positions per block (= one row)
NBLK = N // BLK      # 64
NIDX = 9 * BLK       # gather indices per block
EPS = 1e-5

_NC_CACHE = {}


def _build_nc(has_bias: bool):
    import concourse.bacc as bacc
    import concourse.bass as bass
    import concourse.tile as tile
    import concourse.mybir as mybir
    from concourse import library_config

    f16 = mybir.dt.float16
    f32 = mybir.dt.float32
    i16 = mybir.dt.int16
    Alu = mybir.AluOpType
    Act = mybir.ActivationFunctionType

    nc = bacc.Bacc("TRN2", target_bir_lowering=False, debug=False, num_devices=8,
                   num_swdge_queues=2)

    NTAB = Hp * Wp
    xt2 = nc.dram_tensor("xt2", [NTAB * 1024], f16, kind="ExternalInput")
    xk = nc.dram_tensor("xk", [2, 128, N], f16, kind="ExternalInput")
    idx = nc.dram_tensor("idx", [NBLK, 128, NIDX // 16], i16, kind="ExternalInput")
    w4 = nc.dram_tensor("w4", [NBLK, 128, 36, 32], f16, kind="ExternalInput")
    w4f = nc.dram_tensor("w4f", [NBLK, 128, 8], f32, kind="ExternalInput")
    w0t = nc.dram_tensor("w0t", [2, 128, 257], f16, kind="ExternalInput")
    w1t = nc.dram_tensor("w1t", [2, 128, 257], f16, kind="ExternalInput")
    idmat = nc.dram_tensor("idmat", [128, 128], f16, kind="ExternalInput")
    if has_bias:
        qb = nc.dram_tensor("qb", [128, 257], f32, kind="ExternalInput")
        kb = nc.dram_tensor("kb", [128, 257], f32, kind="ExternalInput")
    o = nc.dram_tensor("o", [128, NBLK], f32, kind="ExternalOutput")

    # gather view: element j = xt2[j*1024 : (j+1)*1024]  (one 2x2 pixel block)
    xt2_view = bass.AP(tensor=xt2[:].tensor, offset=0, ap=[[1024, NTAB], [1, 1024]])

    with tile.TileContext(nc) as tc:
        import contextlib

        with contextlib.ExitStack() as ctx:
            consts = ctx.enter_context(tc.tile_pool(name="consts", bufs=1))
            iopool = ctx.enter_context(tc.tile_pool(name="io", bufs=3))
            gpool = ctx.enter_context(tc.tile_pool(name="gath", bufs=5))
            work = ctx.enter_context(tc.tile_pool(name="work", bufs=2))
            work3 = ctx.enter_context(tc.tile_pool(name="work3", bufs=3))
            pspool = ctx.enter_context(tc.tile_pool(name="ps", bufs=2, space="PSUM"))

            # constants
            w0t_sb = consts.tile([128, 2, 257], f16)
            nc.sync.dma_start(out=w0t_sb, in_=w0t[:, :, :].rearrange("t p o -> p t o"))
            w1t_sb = consts.tile([128, 2, 257], f16)
            nc.sync.dma_start(out=w1t_sb, in_=w1t[:, :, :].rearrange("t p o -> p t o"))
            ident = consts.tile([128, 128], f16)
            nc.sync.dma_start(out=ident, in_=idmat[:, :])
            if has_bias:
                qb_sb = consts.tile([128, 257], f32)
                nc.sync.dma_start(out=qb_sb, in_=qb[:, :])
                kb_sb = consts.tile([128, 257], f32)
                nc.sync.dma_start(out=kb_sb, in_=kb[:, :])

            # per-block scalar accumulators [128 pos, NBLK]
            sqs = consts.tile([128, NBLK], f32, tag="sqs")
            sks = consts.tile([128, NBLK], f32, tag="sks")
            sqks = consts.tile([128, NBLK], f32, tag="sqks")
            sQs = consts.tile([128, NBLK], f32, tag="sQs")
            sKs = consts.tile([128, NBLK], f32, tag="sKs")

            nc.gpsimd.load_library(library_config.mlp)

            pending = None
            for blk in range(NBLK):
                idx_t = iopool.tile([128, NIDX // 16], i16, tag="idx")
                nc.sync.dma_start(out=idx_t, in_=idx[blk])
                w32t = iopool.tile([128, 36, 32], f16, tag="w32t")
                nc.sync.dma_start(out=w32t, in_=w4[blk])
                w4f_t = iopool.tile([128, 8], f32, tag="w4f_t")
                nc.sync.dma_start(out=w4f_t, in_=w4f[blk])
                xk_t = iopool.tile([128, 2, BLK], f16, tag="xk")
                nc.sync.dma_start(
                    out=xk_t, in_=xk[:, :, blk * BLK:(blk + 1) * BLK]
                    .rearrange("t p n -> p t n")
                )
                gat = gpool.tile([128, 9, 1024], f16, tag="gat")
                nc.gpsimd.dma_gather(
                    gat, xt2_view, idx_t, NIDX, NIDX, 1024,
                    single_packet=False, queue_num=blk % 2,
                )

                # t = gat * W: k=0..6 on DVE (weight broadcast folded in
                # via stride-0 middle dim); k=7,8 corner products on the
                # Scalar engine (per-partition scale = per-position weight)
                t_t = work.tile([128, 9, 4, 256], f16, tag="t")
                nc.vector.tensor_tensor(
                    t_t[:, 0:7, :, :].rearrange("p a b (c d) -> p (a b) c d", c=8),
                    gat[:, 0:7, :].rearrange("p a (b c d) -> p (a b) c d", b=4, c=8),
                    w32t[:, 0:28, None, :].to_broadcast([128, 28, 8, 32]),
                    Alu.mult)
                for kk in (7, 8):
                    for r in range(4):
                        nc.scalar.mul(
                            t_t[:, kk, r, :],
                            gat[:, kk, r * 256:(r + 1) * 256],
                            w4f_t[:, (kk - 7) * 4 + r:(kk - 7) * 4 + r + 1])
                # corner pair adds: A = t[:, :, 0:2] + t[:, :, 2:4] -> [128,9,2,256]
                a_t = work.tile([128, 9, 2, 256], f16, tag="a")
                nc.vector.tensor_tensor(
                    a_t, t_t[:, :, 0:2, :], t_t[:, :, 2:4, :], Alu.add)
                # S = A[:, :, 0] + A[:, :, 1] -> [128, 9, 256]
                s_t = work3.tile([128, 9, 256], f16, tag="s")
                nc.vector.tensor_tensor(
                    s_t, a_t[:, :, 0, :], a_t[:, :, 1, :], Alu.add)
                # max tree over the 9 samples
                m1 = work3.tile([128, 4, 256], f16, tag="m1")
                nc.vector.tensor_tensor(m1, s_t[:, 0:4, :], s_t[:, 4:8, :], Alu.max)
                m2 = work3.tile([128, 2, 256], f16, tag="m2")
                nc.vector.tensor_tensor(m2, m1[:, 0:2, :], m1[:, 2:4, :], Alu.max)
                q_t = work3.tile([128, 256], f16, tag="q")
                nc.vector.tensor_tensor(
                    q_t[:, None, :], m2[:, 0:1, :], m2[:, 1:2, :], Alu.max)
                nc.vector.tensor_tensor(
                    q_t[:, None, :], q_t[:, None, :], s_t[:, 8:9, :], Alu.max)

                if pending is not None:
                    pQ, pK, pcol = pending
                    dve_scr = work3.tile([128, 256], f16, tag="dve_scr")
                    nc.vector.scalar_tensor_tensor(
                        dve_scr, pQ[:, 0:256], 0.0, pK, Alu.bypass, Alu.mult,
                        accum_out=sqks[:, pcol],
                    )
                    pending = None

                # transpose q -> qT (c-major) via PE
                qt_ps = pspool.tile([128, 2, 128], f16, tag="qt")
                for t in range(2):
                    nc.tensor.transpose(
                        qt_ps[:, t, :], q_t[:, t * 128:(t + 1) * 128], ident
                    )
                qt_sb = work3.tile([128, 2, 128], f16, tag="qt_sb")
                nc.scalar.copy(qt_sb, qt_ps)

                # Q = qT^T @ w0t  -> [128 pos, 257] (col 256 = sum_o Q)
                Q_ps = pspool.tile([128, 257], f32, tag="Q")
                for t in range(2):
                    nc.tensor.matmul(
                        Q_ps, qt_sb[:, t, :], w0t_sb[:, t, :],
                        start=(t == 0), stop=(t == 1),
                    )
                K_ps = pspool.tile([128, 257], f32, tag="K")
                for t in range(2):
                    nc.tensor.matmul(
                        K_ps, xk_t[:, t, :],
                        w1t_sb[:, t, :], start=(t == 0), stop=(t == 1),
                    )
                if has_bias:
                    nc.vector.tensor_tensor(Q_ps, Q_ps, qb_sb, Alu.add)
                    nc.vector.tensor_tensor(K_ps, K_ps, kb_sb, Alu.add)

                # epilogue reductions (ACT side in-block; the DVE
                # product-accumulate is deferred one block so it never
                # head-of-line blocks the next block's multiply)
                col = slice(blk, blk + 1)
                act_scr = work3.tile([128, 256], f16, tag="act_scr")
                nc.scalar.activation(
                    act_scr, Q_ps[:, 0:256], Act.Square,
                    accum_out=sqs[:, col],
                )
                K_sb = work3.tile([128, 256], f16, tag="K_sb")
                nc.scalar.copy(K_sb, K_ps[:, 0:256])
                nc.scalar.activation(
                    act_scr, K_sb, Act.Square, accum_out=sks[:, col],
                )
                nc.scalar.copy(sQs[:, col], Q_ps[:, 256:257])
                nc.scalar.copy(sKs[:, col], K_ps[:, 256:257])
                pending = (Q_ps, K_sb, col)

            if pending is not None:
                pQ, pK, pcol = pending
                dve_scr = work3.tile([128, 256], f16, tag="dve_scr")
                nc.vector.scalar_tensor_tensor(
                    dve_scr, pQ[:, 0:256], 0.0, pK, Alu.bypass, Alu.mult,
                    accum_out=sqks[:, pcol],
                )

            # final combine over [128, NBLK]
            tmp = consts.tile([128, NBLK], f32, tag="tmp")
            num = consts.tile([128, NBLK], f32, tag="num")
            dq = consts.tile([128, NBLK], f32, tag="dq")
            dk = consts.tile([128, NBLK], f32, tag="dk")
            out_t = consts.tile([128, NBLK], f32, tag="out")
            inv_c = -1.0 / C
            nc.vector.tensor_tensor(tmp, sQs, sKs, Alu.mult)
            nc.vector.scalar_tensor_tensor(num, tmp, inv_c, sqks, Alu.mult, Alu.add)
            nc.vector.tensor_tensor(tmp, sQs, sQs, Alu.mult)
            nc.vector.scalar_tensor_tensor(dq, tmp, inv_c, sqs, Alu.mult, Alu.add)
            nc.vector.tensor_scalar(dq, dq, EPS, None, Alu.add)
            nc.vector.tensor_tensor(tmp, sKs, sKs, Alu.mult)
            nc.vector.scalar_tensor_tensor(dk, tmp, inv_c, sks, Alu.mult, Alu.add)
            nc.vector.tensor_scalar(dk, dk, EPS, None, Alu.add)
            nc.vector.tensor_tensor(tmp, dq, dk, Alu.mult)
            nc.scalar.activation(tmp, tmp, Act.Sqrt)
            nc.vector.reciprocal(tmp, tmp)
            nc.vector.tensor_tensor(out_t, num, tmp, Alu.mult)
            nc.sync.dma_start(out=o[:, :], in_=out_t)

    nc.compile()
    return nc


def _get_nc(has_bias: bool):
    if has_bias not in _NC_CACHE:
        _NC_CACHE[has_bias] = _build_nc(has_bias)
    return _NC_CACHE[has_bias]


def _build_table(x_b):
    """Pair-table for one image: [Hp, Wp, 4, 256] fp16 flat.
    Entry (y, x) = channels of (y,x), (y,x+1), (y+1,x), (y+1,x+1)."""
    xp = np.zeros((Hp, Wp, C), np.float16)
    xp[PAD:PAD + H, PAD:PAD + W, :] = x_b.transpose(1, 2, 0)
    t = np.zeros((Hp, Wp, 4, C), np.float16)
    t[:-1, :-1, 0] = xp[:-1, :-1]
    t[:-1, :-1, 1] = xp[:-1, 1:]
    t[:-1, :-1, 2] = xp[1:, :-1]
    t[:-1, :-1, 3] = xp[1:, 1:]
    return t.reshape(-1)


def _prep_core(x_b, off_b, h0):
    """Host-side per-core prep: pair-table idx [NBLK,128,72] i16,
    corner weights w4 [NBLK,128,36] f16, xk [2,128,N] f16."""
    ys, xs = np.meshgrid(
        np.arange(h0, h0 + ROWS), np.arange(W), indexing="ij"
    )
    ys = ys.reshape(-1).astype(np.float32)   # [N] position y (row-major blocks)
    xs = xs.reshape(-1).astype(np.float32)

    iy = ys.astype(np.int32)
    ix = xs.astype(np.int32)
    k = np.arange(9)
    kh = (k // 3 - 1).astype(np.float32)     # [9]
    kw = (k % 3 - 1).astype(np.float32)

    offy = off_b[2 * k][:, iy, ix]           # [9, N]
    offx = off_b[2 * k + 1][:, iy, ix]
    py = ys[None, :] + kh[:, None] + offy    # [9, N]
    px = xs[None, :] + kw[:, None] + offx
    y0 = np.clip(np.floor(py).astype(np.int32), -PAD, H + PAD - 2)
    x0 = np.clip(np.floor(px).astype(np.int32), -PAD, W + PAD - 2)
    fy = (py - y0).astype(np.float32)        # [9, N]
    fx = (px - x0).astype(np.float32)
    pidx = (y0 + PAD) * Wp + (x0 + PAD)      # [9, N]

    # idx slots: per block, m = k*128 + pos  -> out[pos, k, :]
    slots = pidx.reshape(9, NBLK, BLK).transpose(1, 0, 2)  # [NBLK, 9, BLK]
    wrapped = slots.reshape(NBLK, NIDX // 16, 16).transpose(0, 2, 1)  # [NBLK,16,72]
    idx_np = np.tile(wrapped, (1, 8, 1)).astype(np.int16)  # [NBLK, 128, 72]

    # corner weights [NBLK, 128, 36, 64] f16 (k-major, corner minor, rep-64)
    gy1 = (1.0 - fy) * (1.0 - fx)
    gy2 = (1.0 - fy) * fx
    gy3 = fy * (1.0 - fx)
    gy4 = fy * fx
    w_all = np.stack([gy1, gy2, gy3, gy4], axis=1)  # [9, 4, N]
    w_all = w_all.reshape(9, 4, NBLK, BLK).transpose(2, 3, 0, 1)  # [NBLK,BLK,9,4]
    w4_np = np.ascontiguousarray(np.broadcast_to(
        w_all.reshape(NBLK, 128, 36, 1).astype(np.float16),
        (NBLK, 128, 36, 32)))
    w4f_np = np.ascontiguousarray(
        w_all.reshape(NBLK, 128, 36)[:, :, 28:36].astype(np.float32))

    xk_np = np.ascontiguousarray(
        x_b.reshape(2, 128, H, W)[:, :, h0:h0 + ROWS, :].reshape(2, 128, N)
    ).astype(np.float16)
    return idx_np, w4_np, w4f_np, xk_np


def prep_in_maps(x, offset, w0, b0, w1, b1, has_bias):
    w0t_np = np.concatenate([w0.T, w0.sum(0)[:, None]], 1).astype(np.float16)
    w1t_np = np.concatenate([w1.T, w1.sum(0)[:, None]], 1).astype(np.float16)
    w0t_np = np.ascontiguousarray(w0t_np.reshape(2, 128, 257))
    w1t_np = np.ascontiguousarray(w1t_np.reshape(2, 128, 257))

    in_maps = []
    xt_cache = {}
    for core in range(8):
        b, half = core // 2, core % 2
        h0 = ROWS * half
        if b not in xt_cache:
            xt_cache[b] = _build_table(x[b])
        idx_np, w4_np, w4f_np, xk_np = _prep_core(x[b], offset[b], h0)
        m = {
            "idmat": np.eye(128, dtype=np.float16),
            "xt2": xt_cache[b],
            "xk": xk_np,
            "idx": idx_np,
            "w4": w4_np,
            "w4f": w4f_np,
            "w0t": w0t_np,
            "w1t": w1t_np,
        }
        if has_bias:
            qb_np = np.concatenate([b0, [b0.sum()]]).astype(np.float32)
            kb_np = np.concatenate([b1, [b1.sum()]]).astype(np.float32)
            m["qb"] = np.tile(qb_np[None, :], (128, 1))
            m["kb"] = np.tile(kb_np[None, :], (128, 1))
        in_maps.append(m)
    return in_maps


def kernel(x, offset, w0, b0, w1, b1):
    from concourse.bass_utils import run_bass_kernel_spmd

    x = np.asarray(x, np.float32)
    offset = np.asarray(offset, np.float32)
    w0 = np.asarray(w0, np.float32)
    w1 = np.asarray(w1, np.float32)
    b0 = np.asarray(b0, np.float32)
    b1 = np.asarray(b1, np.float32)

    has_bias = bool(np.any(b0)) or bool(np.any(b1))
    nc = _get_nc(has_bias)
    in_maps = prep_in_maps(x, offset, w0, b0, w1, b1, has_bias)
    res = run_bass_kernel_spmd(nc, in_maps, core_ids=list(range(8)))

    out = np.empty((B, 1, H, W), np.float32)
    for core in range(8):
        b, half = core // 2, core % 2
        h0 = ROWS * half
        o = res.results[core]["o"]  # [128 pos(x), 64 rows]
        out[b, 0, h0:h0 + ROWS, :] = o.T
    return out

